# revision 59
# baseline (speedup 1.0000x reference)
"""Trainium2 Bass kernel for GroupedQueryAttention (anti-causal mask variant).

Reference semantics (B=2, S=2048, D=4096, 32 Q heads, 4 KV heads, dk=128):
  Q = x@Wq, K = x@Wk, V = x@Wv (heads split), GQA repeat KV x8.
  scores = Q K^T / sqrt(dk); mask = triu(ones, k=1); scores = where(mask==0, -1e9, scores)
    -> keeps STRICT UPPER triangle (k > q, anti-causal). Rows with no valid key
       (q == S-1) become a uniform softmax over all S keys.
  out = softmax(scores) @ V; out = out @ Wo.

Sharding: 8 cores, 4 Q heads + their 1 shared KV head per core. Each core
computes a partial out = attn_heads @ Wo_rows_slice; host sums the 8 partials.

Per-core kernel design (bf16 operands, fp32 PSUM accumulation):
  - x is pre-cast to bf16 on the host (inputs stay fp32 at the kernel()
    boundary); x^T tiles via PE transposes (bf16, 1 cycle/row) + DVE/ACT
    copies out of PSUM.
  - Q^T/K^T/V^T projections in [dk, seq] layout (lhsT = bf16 W chunk, FWL).
  - scores computed TRANSPOSED: sT[k, q] = K^T chunk (lhsT) x Q^T (rhs), so
    softmax denominator is a partition-dim sum (ones-matmul) and the AV matmul
    out^T[dk, q] = V chunk (lhsT) x P^T (rhs) accumulates with N=512 and lands
    already transposed for the Wo projection.
  - exp on ACT over CHUNK PAIRS ([128,1024] spanning two PSUM banks), bf16
    out; masking applied POST-exp as cheap bf16 multiplies on the DVE
    (pt *= M01 gives exact zeros, matching exp(-1e9) -> 0). For the LAST q
    block the reference's fully-masked rows need uniform weights, so there
    pt = exp(s)*M01 + exp(-30)*(1-M01), and the skipped blocks' contributions
    are added analytically: r += n_skip*128*exp(-30), out^T += exp(-30)*cumsumV.
"""

import sys
from contextlib import ExitStack

import numpy as np

for _p in ("/opt/trn_rl_repo",):
    if _p not in sys.path:
        sys.path.insert(0, _p)

import bass_rust
import concourse.bass as bass
import concourse.mybir as mybir
import concourse.tile as tile
from concourse.masks import make_identity


def _split_multiwaits(nc):
    """This walrus build encodes at most ONE sem wait per instruction.
    Tile's wait-assignment can attach several; hoist the extras onto fresh
    single-wait NoOps emitted immediately before the instruction on the same
    engine stream. Tile emits instructions in schedule order, so every wait's
    producer precedes the waiting instruction in-stream and the stall cannot
    deadlock."""
    for fn in nc.m.functions:
        for blk in fn.blocks:
            newlist = []
            for ins in blk.instructions:
                si = ins.sync_info
                n = len(si.on_wait) if si is not None else 0
                if n > 1:
                    waits = list(si.on_wait)
                    for j, w in enumerate(waits[:-1]):
                        nop = mybir.InstNoOp(
                            name=f"{ins.name}-hw{j}", engine=ins.engine,
                            ins=[], outs=[],
                            sync_info=bass_rust.SyncInfo(on_wait=[w],
                                                         on_update=[]))
                        nc.register_instruction(nop, overwrite=True)
                        newlist.append(nop)
                    si.on_wait = waits[-1:]
                newlist.append(ins)
            blk.instructions = newlist

B, S, D = 2, 2048, 4096
NQ, NKV, DK = 32, 4, 128
NCORES = 8
HPC = NQ // NCORES          # 4 q heads per core
DKC = HPC * DK              # 512 proj cols per core
SCALE = 1.0 / float(np.sqrt(DK))
MV = 30.0                   # masked logit magnitude (post-scale)
EXP_M = float(np.exp(-MV))
QB = 512                    # q block (matmul moving free dim)
KC = 128                    # k chunk (PE contraction/partition dim)
F32 = mybir.dt.float32
BF16 = mybir.dt.bfloat16
EXP = mybir.ActivationFunctionType.Exp


def build_program(s=S):
    """Build the per-core Bass/Tile program. Same program for all 8 cores
    (SPMD); per-core weight slices are supplied via the input maps."""
    nqb = s // QB            # q blocks
    nkc = s // KC            # k chunks
    nd = D // KC             # D contraction chunks (32)
    nnb = D // QB            # 8 column blocks of Wo

    nc = bass.Bass("TRN2", target_bir_lowering=False, debug=False,
                   num_devices=NCORES)
    xbt = nc.dram_tensor("xbt", [B, D, s], BF16, kind="ExternalInput").ap()
    wq = nc.dram_tensor("wq", [D, DKC], BF16, kind="ExternalInput").ap()
    wk = nc.dram_tensor("wk", [D, DK], BF16, kind="ExternalInput").ap()
    wv = nc.dram_tensor("wv", [D, DK], BF16, kind="ExternalInput").ap()
    wo = nc.dram_tensor("wo", [DKC, D], BF16, kind="ExternalInput").ap()
    m01 = nc.dram_tensor("mask01", [4, KC, QB], BF16, kind="ExternalInput").ap()
    mem = nc.dram_tensor("maskem", [4, KC, QB], BF16, kind="ExternalInput").ap()
    out = nc.dram_tensor("out", [B, s, D], F32, kind="ExternalOutput").ap()

    of = out.rearrange("b s d -> (b s) d")

    with tile.TileContext(nc) as tc, ExitStack() as ctx:
        consts = ctx.enter_context(tc.tile_pool(name="consts", bufs=1))
        ident = consts.tile([128, 128], BF16, name="ident", tag="ident")
        make_identity(nc, ident)
        ones = consts.tile([128, 128], BF16, name="ones", tag="ones")
        nc.vector.memset(ones, 1.0)

        # masks (bf16, applied post-exp)
        m01_t = consts.tile([128, 4, QB], BF16, name="m01_t", tag="m01_t")
        nc.sync.dma_start(out=m01_t, in_=m01.rearrange("d p n -> p d n"))
        mem_t = consts.tile([128, 4, QB], BF16, name="mem_t", tag="mem_t")
        nc.sync.dma_start(out=mem_t, in_=mem.rearrange("d p n -> p d n"))

        # weights: loaded once, reused for both batches
        wpool = ctx.enter_context(tc.tile_pool(name="wqkv", bufs=1))
        wq_t = wpool.tile([128, nd, DKC], BF16, name="wq_t", tag="wq_t")
        nc.sync.dma_start(out=wq_t, in_=wq.rearrange("(c p) n -> p c n", p=128))
        wk_t = wpool.tile([128, nd, DK], BF16, name="wk_t", tag="wk_t")
        nc.sync.dma_start(out=wk_t, in_=wk.rearrange("(c p) n -> p c n", p=128))
        wv_t = wpool.tile([128, nd, DK], BF16, name="wv_t", tag="wv_t")
        nc.sync.dma_start(out=wv_t, in_=wv.rearrange("(c p) n -> p c n", p=128))
        wo_t = wpool.tile([128, HPC, nnb, QB], BF16, name="wo_t", tag="wo_t")
        # wo_t's DMA is issued after the first projection block (it is only
        # needed in the Wo phase) to keep startup HBM bandwidth for x loads

        # output staging: hoisted (stable address) so the next batch's xq
        # allocation never aliases it and waits on store DMAs
        stpool = ctx.enter_context(tc.tile_pool(name="ostage", bufs=2))
        # x^T loads: hoisted pool (stable buffers -> cross-batch loads only
        # wait on their own previous consumer, not on batch-tail aliases)
        xqp = ctx.enter_context(tc.tile_pool(name="xqld", bufs=2))

        nskip = 4 * (nqb - 1)   # fully-masked chunks of the last q block

        for b in range(B):
            with ExitStack() as bctx:
                bpool = bctx.enter_context(tc.tile_pool(name=f"bp{b}", bufs=1))
                qt = [bpool.tile([128, s], BF16, name=f"qt{b}_{h}", tag=f"qt{h}")
                      for h in range(HPC)]
                kt = bpool.tile([128, s], BF16, name=f"kt{b}", tag="kt")
                vt = bpool.tile([128, s], BF16, name=f"vt{b}", tag="vt")
                vn = bpool.tile([128, s], BF16, name=f"vn{b}", tag="vn")
                cv = bpool.tile([128, 1], F32, name=f"cv{b}", tag="cv")

                # ---------- projection phase: Q^T, K^T, V^T ----------
                # x^T comes pre-transposed from the host (xbt = [B, D, s]);
                # one bulk DMA per q block feeds 192 dependency-free matmuls.
                xbtr = xbt[b].rearrange("(c p) s -> p c s", p=128)
                ndq4 = nd // 4       # D chunks per quarter-load
                with ExitStack() as pctx:
                    qpool = pctx.enter_context(
                        tc.tile_pool(name="qpsum", bufs=1, space="PSUM"))
                    kvpool = pctx.enter_context(
                        tc.tile_pool(name="kvpsum", bufs=1, space="PSUM"))

                    for qb in range(nqb):
                        sl = slice(qb * QB, (qb + 1) * QB)
                        pq = [qpool.tile([128, QB], F32, name=f"pq{h}", tag=f"pq{h}")
                              for h in range(HPC)]
                        pk = kvpool.tile([128, QB], F32, name="pk", tag="pk")
                        pv = kvpool.tile([128, QB], F32, name="pv", tag="pv")
                        for dqi in range(4):
                            xq = xqp.tile([128, ndq4, QB], BF16, name="xq",
                                          tag="xq")
                            # gpsimd SWDGE queue: its stream is free by
                            # mid-attention, so next-batch loads enqueue
                            # early instead of behind the store flood
                            nc.gpsimd.dma_start(
                                out=xq,
                                in_=xbtr[:, dqi * ndq4:(dqi + 1) * ndq4, sl])
                            for dj in range(ndq4):
                                dc = dqi * ndq4 + dj
                                st = dc == 0
                                sp = dc == nd - 1
                                for h in range(HPC):
                                    nc.tensor.matmul(
                                        pq[h], wq_t[:, dc, h * 128:(h + 1) * 128],
                                        xq[:, dj, :], start=st, stop=sp)
                                nc.tensor.matmul(pk, wk_t[:, dc, :],
                                                 xq[:, dj, :], start=st, stop=sp)
                                nc.tensor.matmul(pv, wv_t[:, dc, :],
                                                 xq[:, dj, :], start=st, stop=sp)
                        for h in range(HPC):
                            if h % 2 == 0:
                                nc.vector.tensor_copy(qt[h][:, sl], pq[h])
                            else:
                                nc.scalar.copy(qt[h][:, sl], pq[h])
                        nc.vector.tensor_copy(kt[:, sl], pk)
                        nc.scalar.copy(vt[:, sl], pv)
                    if b == 0:
                        nc.sync.dma_start(
                            out=wo_t,
                            in_=wo.rearrange("(c p) (nb n) -> p c nb n",
                                             p=128, n=QB))

                # ---------- V^T -> V natural; cv = exp(-30)*cumsum(V) ------
                with ExitStack() as vctx:
                    vpsum = vctx.enter_context(
                        tc.tile_pool(name="vtpsum", bufs=2, space="PSUM"))
                    for kc in range(nkc):
                        pvt = vpsum.tile([128, 128], BF16, name="pvt", tag="pvt")
                        nc.tensor.transpose(
                            pvt, vt[:, kc * 128:(kc + 1) * 128], ident)
                        nc.vector.tensor_copy(vn[:, kc * 128:(kc + 1) * 128], pvt)
                    if nskip > 0:
                        cps = vctx.enter_context(
                            tc.tile_pool(name="cvpsum", bufs=1, space="PSUM"))
                        pc = cps.tile([128, 8], F32, name="pc", tag="pc")
                        for i in range(nskip):
                            nc.tensor.matmul(
                                pc, vn[:, i * 128:(i + 1) * 128], ones[:, 0:8],
                                start=(i == 0), stop=(i == nskip - 1))
                        nc.scalar.mul(cv, pc[:, 0:1], EXP_M)

                # ---------- attention + output projection (fused) ----------
                # qb-major: all 4 heads of a q block finish together, so the
                # Wo matmuls for that block can start one group behind the
                # attention stream. PSUM: scores 2 + po/pr 4 + Wo 2 = 8 banks.
                apool = bctx.enter_context(tc.tile_pool(name=f"att{b}", bufs=1))
                att = [apool.tile([128, s], BF16, name=f"att{b}_{h}", tag=f"att{h}")
                       for h in range(HPC)]
                # long qb0 first: generates Wo filler before the short,
                # drain-bound last block runs
                qb_order = [0, nqb - 1] + list(range(1, nqb - 1))
                with ExitStack() as actx:
                    aps = actx.enter_context(
                        tc.tile_pool(name="atpsum", bufs=2, space="PSUM"))
                    sps = actx.enter_context(
                        tc.tile_pool(name="scpsum", bufs=2, space="PSUM"))
                    opsum = actx.enter_context(
                        tc.tile_pool(name="opsum", bufs=2, space="PSUM"))
                    spool = actx.enter_context(tc.tile_pool(name="attsb", bufs=2))
                    ptp2 = actx.enter_context(tc.tile_pool(name="ptsb", bufs=6))

                    for qb in qb_order:
                        last = qb == nqb - 1
                        qsl = slice(qb * QB, (qb + 1) * QB)
                        kcs = list(range(4 * qb, nkc))
                        for h in range(HPC):
                            po = aps.tile([128, QB], F32, name="po", tag="po")
                            pr = aps.tile([128, QB], F32, name="pr", tag="pr")
                            for i, kc in enumerate(kcs):
                                ps = sps.tile([128, QB], F32, name="ps", tag="ps")
                                nc.tensor.matmul(
                                    ps, kt[:, kc * 128:(kc + 1) * 128],
                                    qt[h][:, qsl], start=True, stop=True)
                                pt = ptp2.tile([128, QB], BF16, name="pt",
                                               tag="pt")
                                nc.scalar.activation(pt, ps, EXP, scale=SCALE)
                                d = kc - 4 * qb
                                if d < 4:
                                    # POOL: keeps DVE free for reciprocal
                                    nc.gpsimd.tensor_mul(pt, pt, m01_t[:, d, :])
                                    if last:
                                        nc.gpsimd.tensor_add(pt, pt,
                                                             mem_t[:, d, :])
                                nc.tensor.matmul(
                                    po, vn[:, kc * 128:(kc + 1) * 128], pt,
                                    start=(i == 0), stop=(i == len(kcs) - 1))
                                nc.tensor.matmul(
                                    pr, ones, pt,
                                    start=(i == 0), stop=(i == len(kcs) - 1))
                            # drain po to SBUF fast (frees its PSUM bank in
                            # ~1us); reciprocal reads pr straight from PSUM
                            # (saves a DVE copy; bank held +3.4us is fine at
                            # the 2-group-deep pipeline cadence)
                            pos = spool.tile([128, QB], F32, name="pos", tag="pos")
                            nc.scalar.copy(pos, po)
                            rr = spool.tile([128, QB], F32, name="rr", tag="rr")
                            if last and nskip > 0:
                                prs = spool.tile([128, QB], F32, name="prs",
                                                 tag="prs")
                                nc.vector.tensor_scalar_add(
                                    prs, pr, float(nskip * 128 * EXP_M))
                                nc.vector.reciprocal(rr, prs)
                            else:
                                nc.vector.reciprocal(rr, pr)
                            if last and nskip > 0:
                                tno = spool.tile([128, QB], F32, name="tno",
                                                 tag="tno")
                                nc.vector.tensor_scalar_add(tno, pos, cv)
                                nc.vector.tensor_mul(att[h][:, qsl], tno, rr)
                            else:
                                nc.vector.tensor_mul(att[h][:, qsl], pos, rr)

                    # Wo, chasing the attention stream one q block behind
                    for qb in qb_order:
                        for qti in range(qb * 4, qb * 4 + 4):
                            row0 = b * s + qti * 128
                            for hf in range(2):
                                stg = stpool.tile([128, D // 2], F32,
                                                  name="stg", tag="stg")
                                for nj in range(nnb // 2):
                                    nb = hf * (nnb // 2) + nj
                                    po2 = opsum.tile([128, QB], F32, name="po2",
                                                     tag="po2")
                                    for c in range(HPC):
                                        nc.tensor.matmul(
                                            po2,
                                            att[c][:, qti * 128:(qti + 1) * 128],
                                            wo_t[:, c, nb, :],
                                            start=(c == 0), stop=(c == HPC - 1))
                                    osl = slice(nj * QB, (nj + 1) * QB)
                                    if nb % 2 == 0:
                                        nc.vector.tensor_copy(stg[:, osl], po2)
                                    else:
                                        nc.scalar.copy(stg[:, osl], po2)
                                # column-split stores spread across DMA queues
                                for oc in range(2):
                                    co = hf * (D // 2) + oc * (D // 4)
                                    nc.sync.dma_start(
                                        out=of[row0:row0 + 128,
                                               co:co + D // 4],
                                        in_=stg[:, oc * (D // 4):
                                                (oc + 1) * (D // 4)])
    _split_multiwaits(nc)
    return nc


def make_masks():
    import ml_dtypes

    bf = ml_dtypes.bfloat16
    r = np.arange(KC)[:, None]
    c = np.arange(QB)[None, :]
    valid = [(r + 128 * d) > c for d in range(4)]   # k > q within block
    m01 = np.stack([v.astype(np.float32) for v in valid]).astype(bf)
    mem = np.stack([np.where(v, 0.0, EXP_M) for v in valid]).astype(bf)
    return m01, mem


_PROG = {}


def _get_program(s=S):
    if s not in _PROG:
        _PROG[s] = build_program(s)
    return _PROG[s]


def core_in_map(c, x, Wq, Wk, Wv, Wo, _shared={}):
    import ml_dtypes

    bf = ml_dtypes.bfloat16
    xid = id(x)
    if _shared.get("xid") != xid:
        _shared["xid"] = xid
        _shared["xbt"] = np.ascontiguousarray(
            np.asarray(x, dtype=np.float32).astype(bf).transpose(0, 2, 1))
        _shared["m01"], _shared["mem"] = make_masks()
    h0 = c * HPC
    kv = (c * HPC) // (NQ // NKV)
    return {
        "xbt": _shared["xbt"],
        "wq": np.ascontiguousarray(
            np.asarray(Wq, np.float32)[:, h0 * DK:(h0 + HPC) * DK].astype(bf)),
        "wk": np.ascontiguousarray(
            np.asarray(Wk, np.float32)[:, kv * DK:(kv + 1) * DK].astype(bf)),
        "wv": np.ascontiguousarray(
            np.asarray(Wv, np.float32)[:, kv * DK:(kv + 1) * DK].astype(bf)),
        "wo": np.ascontiguousarray(
            np.asarray(Wo, np.float32)[h0 * DK:(h0 + HPC) * DK, :].astype(bf)),
        "mask01": _shared["m01"],
        "maskem": _shared["mem"],
    }


def kernel(x, Wq, Wk, Wv, Wo, **kw):
    from concourse.bass_utils import run_bass_kernel_spmd

    nc = _get_program(np.asarray(x).shape[1])
    in_maps = [core_in_map(c, x, Wq, Wk, Wv, Wo) for c in range(NCORES)]
    res = run_bass_kernel_spmd(nc, in_maps, core_ids=list(range(NCORES)), **kw)
    acc = np.zeros(np.asarray(x).shape, np.float64)
    for r in res.results:
        acc += r["out"]
    return acc.astype(np.float32)


# revision 61
# speedup vs baseline: 1.0753x; 1.0753x over previous
"""Trainium2 Bass kernel for GroupedQueryAttention (anti-causal mask variant).

Reference semantics (B=2, S=2048, D=4096, 32 Q heads, 4 KV heads, dk=128):
  Q = x@Wq, K = x@Wk, V = x@Wv (heads split), GQA repeat KV x8.
  scores = Q K^T / sqrt(dk); mask = triu(ones, k=1); scores = where(mask==0, -1e9, scores)
    -> keeps STRICT UPPER triangle (k > q, anti-causal). Rows with no valid key
       (q == S-1) become a uniform softmax over all S keys.
  out = softmax(scores) @ V; out = out @ Wo.

Sharding: 8 cores, 4 Q heads + their 1 shared KV head per core. Each core
computes a partial out = attn_heads @ Wo_rows_slice; host sums the 8 partials.

Per-core kernel design (bf16 operands, fp32 PSUM accumulation):
  - x is pre-cast to bf16 on the host (inputs stay fp32 at the kernel()
    boundary); x^T tiles via PE transposes (bf16, 1 cycle/row) + DVE/ACT
    copies out of PSUM.
  - Q^T/K^T/V^T projections in [dk, seq] layout (lhsT = bf16 W chunk, FWL).
  - scores computed TRANSPOSED: sT[k, q] = K^T chunk (lhsT) x Q^T (rhs), so
    softmax denominator is a partition-dim sum (ones-matmul) and the AV matmul
    out^T[dk, q] = V chunk (lhsT) x P^T (rhs) accumulates with N=512 and lands
    already transposed for the Wo projection.
  - exp on ACT over CHUNK PAIRS ([128,1024] spanning two PSUM banks), bf16
    out; masking applied POST-exp as cheap bf16 multiplies on the DVE
    (pt *= M01 gives exact zeros, matching exp(-1e9) -> 0). For the LAST q
    block the reference's fully-masked rows need uniform weights, so there
    pt = exp(s)*M01 + exp(-30)*(1-M01), and the skipped blocks' contributions
    are added analytically: r += n_skip*128*exp(-30), out^T += exp(-30)*cumsumV.
"""

import sys
from contextlib import ExitStack

import numpy as np

for _p in ("/opt/trn_rl_repo",):
    if _p not in sys.path:
        sys.path.insert(0, _p)

import bass_rust
import concourse.bass as bass
import concourse.mybir as mybir
import concourse.tile as tile
from concourse.masks import make_identity


def _split_multiwaits(nc):
    """This walrus build encodes at most ONE sem wait per instruction.
    Tile's wait-assignment can attach several; hoist the extras onto fresh
    single-wait NoOps emitted immediately before the instruction on the same
    engine stream. Tile emits instructions in schedule order, so every wait's
    producer precedes the waiting instruction in-stream and the stall cannot
    deadlock."""
    for fn in nc.m.functions:
        for blk in fn.blocks:
            newlist = []
            for ins in blk.instructions:
                si = ins.sync_info
                n = len(si.on_wait) if si is not None else 0
                if n > 1:
                    waits = list(si.on_wait)
                    for j, w in enumerate(waits[:-1]):
                        nop = mybir.InstNoOp(
                            name=f"{ins.name}-hw{j}", engine=ins.engine,
                            ins=[], outs=[],
                            sync_info=bass_rust.SyncInfo(on_wait=[w],
                                                         on_update=[]))
                        nc.register_instruction(nop, overwrite=True)
                        newlist.append(nop)
                    si.on_wait = waits[-1:]
                newlist.append(ins)
            blk.instructions = newlist

B, S, D = 2, 2048, 4096
NQ, NKV, DK = 32, 4, 128
NCORES = 8
HPC = NQ // NCORES          # 4 q heads per core
DKC = HPC * DK              # 512 proj cols per core
SCALE = 1.0 / float(np.sqrt(DK))
MV = 30.0                   # masked logit magnitude (post-scale)
EXP_M = float(np.exp(-MV))
QB = 512                    # q block (matmul moving free dim)
KC = 128                    # k chunk (PE contraction/partition dim)
F32 = mybir.dt.float32
BF16 = mybir.dt.bfloat16
EXP = mybir.ActivationFunctionType.Exp


def build_program(s=S):
    """Build the per-core Bass/Tile program. Same program for all 8 cores
    (SPMD); per-core weight slices are supplied via the input maps."""
    nqb = s // QB            # q blocks
    nkc = s // KC            # k chunks
    nd = D // KC             # D contraction chunks (32)
    nnb = D // QB            # 8 column blocks of Wo

    nc = bass.Bass("TRN2", target_bir_lowering=False, debug=False,
                   num_devices=NCORES)
    xbt = nc.dram_tensor("xbt", [B, D, s], BF16, kind="ExternalInput").ap()
    wq = nc.dram_tensor("wq", [D, DKC], BF16, kind="ExternalInput").ap()
    wk = nc.dram_tensor("wk", [D, DK], BF16, kind="ExternalInput").ap()
    wv = nc.dram_tensor("wv", [D, DK], BF16, kind="ExternalInput").ap()
    wo = nc.dram_tensor("wo", [DKC, D], BF16, kind="ExternalInput").ap()
    m01 = nc.dram_tensor("mask01", [4, KC, QB], BF16, kind="ExternalInput").ap()
    mem = nc.dram_tensor("maskem", [4, KC, QB], BF16, kind="ExternalInput").ap()
    out = nc.dram_tensor("out", [B, s, D], F32, kind="ExternalOutput").ap()

    of = out.rearrange("b s d -> (b s) d")

    with tile.TileContext(nc) as tc, ExitStack() as ctx:
        consts = ctx.enter_context(tc.tile_pool(name="consts", bufs=1))
        ident = consts.tile([128, 128], BF16, name="ident", tag="ident")
        make_identity(nc, ident)
        ones = consts.tile([128, 128], BF16, name="ones", tag="ones")
        nc.vector.memset(ones, 1.0)

        # masks (bf16, applied post-exp)
        m01_t = consts.tile([128, 4, QB], BF16, name="m01_t", tag="m01_t")
        nc.sync.dma_start(out=m01_t, in_=m01.rearrange("d p n -> p d n"))
        mem_t = consts.tile([128, 4, QB], BF16, name="mem_t", tag="mem_t")
        nc.sync.dma_start(out=mem_t, in_=mem.rearrange("d p n -> p d n"))

        # weights: loaded once, reused for both batches
        wpool = ctx.enter_context(tc.tile_pool(name="wqkv", bufs=1))
        wq_t = wpool.tile([128, nd, DKC], BF16, name="wq_t", tag="wq_t")
        nc.sync.dma_start(out=wq_t, in_=wq.rearrange("(c p) n -> p c n", p=128))
        wk_t = wpool.tile([128, nd, DK], BF16, name="wk_t", tag="wk_t")
        nc.sync.dma_start(out=wk_t, in_=wk.rearrange("(c p) n -> p c n", p=128))
        wv_t = wpool.tile([128, nd, DK], BF16, name="wv_t", tag="wv_t")
        nc.sync.dma_start(out=wv_t, in_=wv.rearrange("(c p) n -> p c n", p=128))
        wo_t = wpool.tile([128, HPC, nnb, QB], BF16, name="wo_t", tag="wo_t")
        # wo_t's DMA is issued after the first projection block (it is only
        # needed in the Wo phase) to keep startup HBM bandwidth for x loads

        # output staging: hoisted (stable address) so the next batch's xq
        # allocation never aliases it and waits on store DMAs
        stpool = ctx.enter_context(tc.tile_pool(name="ostage", bufs=2))
        # x^T loads: hoisted pool (stable buffers -> cross-batch loads only
        # wait on their own previous consumer, not on batch-tail aliases)
        xqp = ctx.enter_context(tc.tile_pool(name="xqld", bufs=2))

        nskip = 4 * (nqb - 1)   # fully-masked chunks of the last q block

        for b in range(B):
            with ExitStack() as bctx:
                bpool = bctx.enter_context(tc.tile_pool(name=f"bp{b}", bufs=1))
                qt = [bpool.tile([128, s], BF16, name=f"qt{b}_{h}", tag=f"qt{h}")
                      for h in range(HPC)]
                kt = bpool.tile([128, s], BF16, name=f"kt{b}", tag="kt")
                vt = bpool.tile([128, s], BF16, name=f"vt{b}", tag="vt")
                vn = bpool.tile([128, s], BF16, name=f"vn{b}", tag="vn")
                cv = bpool.tile([128, 1], F32, name=f"cv{b}", tag="cv")

                # ---------- projection phase: Q^T, K^T, V^T ----------
                # x^T comes pre-transposed from the host (xbt = [B, D, s]);
                # one bulk DMA per q block feeds 192 dependency-free matmuls.
                xbtr = xbt[b].rearrange("(c p) s -> p c s", p=128)
                ndq4 = nd // 4       # D chunks per quarter-load
                with ExitStack() as pctx:
                    qpool = pctx.enter_context(
                        tc.tile_pool(name="qpsum", bufs=1, space="PSUM"))
                    kvpool = pctx.enter_context(
                        tc.tile_pool(name="kvpsum", bufs=1, space="PSUM"))

                    for qb in range(nqb):
                        sl = slice(qb * QB, (qb + 1) * QB)
                        pq = [qpool.tile([128, QB], F32, name=f"pq{h}", tag=f"pq{h}")
                              for h in range(HPC)]
                        pk = kvpool.tile([128, QB], F32, name="pk", tag="pk")
                        pv = kvpool.tile([128, QB], F32, name="pv", tag="pv")
                        for dqi in range(4):
                            xq = xqp.tile([128, ndq4, QB], BF16, name="xq",
                                          tag="xq")
                            # gpsimd SWDGE queue: its stream is free by
                            # mid-attention, so next-batch loads enqueue
                            # early instead of behind the store flood
                            nc.gpsimd.dma_start(
                                out=xq,
                                in_=xbtr[:, dqi * ndq4:(dqi + 1) * ndq4, sl])
                            for dj in range(ndq4):
                                dc = dqi * ndq4 + dj
                                st = dc == 0
                                sp = dc == nd - 1
                                for h in range(HPC):
                                    nc.tensor.matmul(
                                        pq[h], wq_t[:, dc, h * 128:(h + 1) * 128],
                                        xq[:, dj, :], start=st, stop=sp)
                                nc.tensor.matmul(pk, wk_t[:, dc, :],
                                                 xq[:, dj, :], start=st, stop=sp)
                                nc.tensor.matmul(pv, wv_t[:, dc, :],
                                                 xq[:, dj, :], start=st, stop=sp)
                        for h in range(HPC):
                            if h % 2 == 0:
                                nc.vector.tensor_copy(qt[h][:, sl], pq[h])
                            else:
                                nc.scalar.copy(qt[h][:, sl], pq[h])
                        nc.vector.tensor_copy(kt[:, sl], pk)
                        nc.scalar.copy(vt[:, sl], pv)
                    if b == 0:
                        nc.sync.dma_start(
                            out=wo_t,
                            in_=wo.rearrange("(c p) (nb n) -> p c nb n",
                                             p=128, n=QB))

                # ---------- V^T -> V natural; cv = exp(-30)*cumsum(V) ------
                with ExitStack() as vctx:
                    vpsum = vctx.enter_context(
                        tc.tile_pool(name="vtpsum", bufs=2, space="PSUM"))
                    for kc in range(nkc):
                        pvt = vpsum.tile([128, 128], BF16, name="pvt", tag="pvt")
                        nc.tensor.transpose(
                            pvt, vt[:, kc * 128:(kc + 1) * 128], ident)
                        nc.vector.tensor_copy(vn[:, kc * 128:(kc + 1) * 128], pvt)
                    if nskip > 0:
                        cps = vctx.enter_context(
                            tc.tile_pool(name="cvpsum", bufs=1, space="PSUM"))
                        pc = cps.tile([128, 8], F32, name="pc", tag="pc")
                        for i in range(nskip):
                            nc.tensor.matmul(
                                pc, vn[:, i * 128:(i + 1) * 128], ones[:, 0:8],
                                start=(i == 0), stop=(i == nskip - 1))
                        nc.scalar.mul(cv, pc[:, 0:1], EXP_M)

                # ---------- attention + output projection (fused) ----------
                # qb-major: all 4 heads of a q block finish together, so the
                # Wo matmuls for that block can start one group behind the
                # attention stream. PSUM: scores 2 + po/pr 4 + Wo 2 = 8 banks.
                apool = bctx.enter_context(tc.tile_pool(name=f"att{b}", bufs=1))
                att = [apool.tile([128, s], BF16, name=f"att{b}_{h}", tag=f"att{h}")
                       for h in range(HPC)]
                qb_order = [nqb - 1] + list(range(nqb - 1))
                with ExitStack() as actx:
                    aps = actx.enter_context(
                        tc.tile_pool(name="atpsum", bufs=2, space="PSUM"))
                    sps = actx.enter_context(
                        tc.tile_pool(name="scpsum", bufs=2, space="PSUM"))
                    opsum = actx.enter_context(
                        tc.tile_pool(name="opsum", bufs=2, space="PSUM"))
                    spool = actx.enter_context(tc.tile_pool(name="attsb", bufs=2))
                    ptp2 = actx.enter_context(tc.tile_pool(name="ptsb", bufs=6))

                    for qb in qb_order:
                        last = qb == nqb - 1
                        qsl = slice(qb * QB, (qb + 1) * QB)
                        kcs = list(range(4 * qb, nkc))
                        if last:
                            # all chunks diagonal; keep full width (masked
                            # entries must carry exp(-30) for uniform rows)
                            chunks = [(kc, QB) for kc in kcs]
                        else:
                            # off-diagonal (full-width) chunks first so the
                            # truncated diagonal chunks accumulate onto
                            # initialized PSUM columns. Diagonal chunk d has
                            # valid entries only in its first (d+1)*KC q
                            # columns; the rest are exact zeros post-mask, so
                            # skipping them is exact.
                            chunks = ([(kc, QB) for kc in kcs[4:]]
                                      + [(kcs[d], (d + 1) * KC)
                                         for d in range(4)])
                        for h in range(HPC):
                            po = aps.tile([128, QB], F32, name="po", tag="po")
                            pr = aps.tile([128, QB], F32, name="pr", tag="pr")
                            for i, (kc, w) in enumerate(chunks):
                                q0 = qb * QB
                                ps = sps.tile([128, QB], F32, name="ps", tag="ps")
                                nc.tensor.matmul(
                                    ps[:, 0:w], kt[:, kc * 128:(kc + 1) * 128],
                                    qt[h][:, q0:q0 + w], start=True, stop=True)
                                pt = ptp2.tile([128, QB], BF16, name="pt",
                                               tag="pt")
                                nc.scalar.activation(pt[:, 0:w], ps[:, 0:w],
                                                     EXP, scale=SCALE)
                                d = kc - 4 * qb
                                if d < 4:
                                    # POOL: keeps DVE free for reciprocal
                                    nc.gpsimd.tensor_mul(pt[:, 0:w], pt[:, 0:w],
                                                         m01_t[:, d, 0:w])
                                    if last:
                                        nc.gpsimd.tensor_add(pt, pt,
                                                             mem_t[:, d, :])
                                nc.tensor.matmul(
                                    po[:, 0:w],
                                    vn[:, kc * 128:(kc + 1) * 128], pt[:, 0:w],
                                    start=(i == 0),
                                    stop=(i == len(chunks) - 1))
                                nc.tensor.matmul(
                                    pr[:, 0:w], ones, pt[:, 0:w],
                                    start=(i == 0),
                                    stop=(i == len(chunks) - 1))
                            # drain po/pr to SBUF fast (frees PSUM banks in
                            # ~1us); reciprocal runs later, off the PE path
                            pos = spool.tile([128, QB], F32, name="pos", tag="pos")
                            prs = spool.tile([128, QB], F32, name="prs", tag="prs")
                            nc.scalar.copy(pos, po)
                            if last and nskip > 0:
                                nc.vector.tensor_scalar_add(
                                    prs, pr, float(nskip * 128 * EXP_M))
                            else:
                                nc.vector.tensor_copy(prs, pr)
                            rr = spool.tile([128, QB], F32, name="rr", tag="rr")
                            nc.vector.reciprocal(rr, prs)
                            if last and nskip > 0:
                                tno = spool.tile([128, QB], F32, name="tno",
                                                 tag="tno")
                                nc.vector.tensor_scalar_add(tno, pos, cv)
                                nc.vector.tensor_mul(att[h][:, qsl], tno, rr)
                            else:
                                nc.vector.tensor_mul(att[h][:, qsl], pos, rr)

                    # Wo, chasing the attention stream one q block behind
                    for qb in qb_order:
                        for qti in range(qb * 4, qb * 4 + 4):
                            row0 = b * s + qti * 128
                            for hf in range(2):
                                stg = stpool.tile([128, D // 2], F32,
                                                  name="stg", tag="stg")
                                for nj in range(nnb // 2):
                                    nb = hf * (nnb // 2) + nj
                                    po2 = opsum.tile([128, QB], F32, name="po2",
                                                     tag="po2")
                                    for c in range(HPC):
                                        nc.tensor.matmul(
                                            po2,
                                            att[c][:, qti * 128:(qti + 1) * 128],
                                            wo_t[:, c, nb, :],
                                            start=(c == 0), stop=(c == HPC - 1))
                                    osl = slice(nj * QB, (nj + 1) * QB)
                                    if nb % 2 == 0:
                                        nc.vector.tensor_copy(stg[:, osl], po2)
                                    else:
                                        nc.scalar.copy(stg[:, osl], po2)
                                # column-split stores spread across DMA queues
                                for oc in range(2):
                                    co = hf * (D // 2) + oc * (D // 4)
                                    nc.sync.dma_start(
                                        out=of[row0:row0 + 128,
                                               co:co + D // 4],
                                        in_=stg[:, oc * (D // 4):
                                                (oc + 1) * (D // 4)])
    _split_multiwaits(nc)
    return nc


def make_masks():
    import ml_dtypes

    bf = ml_dtypes.bfloat16
    r = np.arange(KC)[:, None]
    c = np.arange(QB)[None, :]
    valid = [(r + 128 * d) > c for d in range(4)]   # k > q within block
    m01 = np.stack([v.astype(np.float32) for v in valid]).astype(bf)
    mem = np.stack([np.where(v, 0.0, EXP_M) for v in valid]).astype(bf)
    return m01, mem


_PROG = {}


def _get_program(s=S):
    if s not in _PROG:
        _PROG[s] = build_program(s)
    return _PROG[s]


def core_in_map(c, x, Wq, Wk, Wv, Wo, _shared={}):
    import ml_dtypes

    bf = ml_dtypes.bfloat16
    xid = id(x)
    if _shared.get("xid") != xid:
        _shared["xid"] = xid
        _shared["xbt"] = np.ascontiguousarray(
            np.asarray(x, dtype=np.float32).astype(bf).transpose(0, 2, 1))
        _shared["m01"], _shared["mem"] = make_masks()
    h0 = c * HPC
    kv = (c * HPC) // (NQ // NKV)
    return {
        "xbt": _shared["xbt"],
        "wq": np.ascontiguousarray(
            np.asarray(Wq, np.float32)[:, h0 * DK:(h0 + HPC) * DK].astype(bf)),
        "wk": np.ascontiguousarray(
            np.asarray(Wk, np.float32)[:, kv * DK:(kv + 1) * DK].astype(bf)),
        "wv": np.ascontiguousarray(
            np.asarray(Wv, np.float32)[:, kv * DK:(kv + 1) * DK].astype(bf)),
        "wo": np.ascontiguousarray(
            np.asarray(Wo, np.float32)[h0 * DK:(h0 + HPC) * DK, :].astype(bf)),
        "mask01": _shared["m01"],
        "maskem": _shared["mem"],
    }


def kernel(x, Wq, Wk, Wv, Wo, **kw):
    from concourse.bass_utils import run_bass_kernel_spmd

    nc = _get_program(np.asarray(x).shape[1])
    in_maps = [core_in_map(c, x, Wq, Wk, Wv, Wo) for c in range(NCORES)]
    res = run_bass_kernel_spmd(nc, in_maps, core_ids=list(range(NCORES)), **kw)
    acc = np.zeros(np.asarray(x).shape, np.float64)
    for r in res.results:
        acc += r["out"]
    return acc.astype(np.float32)


# revision 70
# speedup vs baseline: 1.1037x; 1.0264x over previous
"""Trainium2 Bass kernel for GroupedQueryAttention (anti-causal mask variant).

Reference semantics (B=2, S=2048, D=4096, 32 Q heads, 4 KV heads, dk=128):
  Q = x@Wq, K = x@Wk, V = x@Wv (heads split), GQA repeat KV x8.
  scores = Q K^T / sqrt(dk); mask = triu(ones, k=1); scores = where(mask==0, -1e9, scores)
    -> keeps STRICT UPPER triangle (k > q, anti-causal). Rows with no valid key
       (q == S-1) become a uniform softmax over all S keys.
  out = softmax(scores) @ V; out = out @ Wo.

Sharding: 8 cores, 4 Q heads + their 1 shared KV head per core. Each core
computes a partial out = attn_heads @ Wo_rows_slice; host sums the 8 partials.

Per-core kernel design (bf16 operands, fp32 PSUM accumulation):
  - x is pre-cast to bf16 on the host (inputs stay fp32 at the kernel()
    boundary); x^T tiles via PE transposes (bf16, 1 cycle/row) + DVE/ACT
    copies out of PSUM.
  - Q^T/K^T/V^T projections in [dk, seq] layout (lhsT = bf16 W chunk, FWL).
  - scores computed TRANSPOSED: sT[k, q] = K^T chunk (lhsT) x Q^T (rhs), so
    softmax denominator is a partition-dim sum (ones-matmul) and the AV matmul
    out^T[dk, q] = V chunk (lhsT) x P^T (rhs) accumulates with N=512 and lands
    already transposed for the Wo projection.
  - exp on ACT over CHUNK PAIRS ([128,1024] spanning two PSUM banks), bf16
    out; masking applied POST-exp as cheap bf16 multiplies on the DVE
    (pt *= M01 gives exact zeros, matching exp(-1e9) -> 0). For the LAST q
    block the reference's fully-masked rows need uniform weights, so there
    pt = exp(s)*M01 + exp(-30)*(1-M01), and the skipped blocks' contributions
    are added analytically: r += n_skip*128*exp(-30), out^T += exp(-30)*cumsumV.
"""

import sys
from contextlib import ExitStack

import numpy as np

for _p in ("/opt/trn_rl_repo",):
    if _p not in sys.path:
        sys.path.insert(0, _p)

import bass_rust
import concourse.bass as bass
import concourse.mybir as mybir
import concourse.tile as tile
from concourse.masks import make_identity


def _split_multiwaits(nc):
    """This walrus build encodes at most ONE sem wait per instruction.
    Tile's wait-assignment can attach several; hoist the extras onto fresh
    single-wait NoOps emitted immediately before the instruction on the same
    engine stream. Tile emits instructions in schedule order, so every wait's
    producer precedes the waiting instruction in-stream and the stall cannot
    deadlock."""
    for fn in nc.m.functions:
        for blk in fn.blocks:
            newlist = []
            for ins in blk.instructions:
                si = ins.sync_info
                n = len(si.on_wait) if si is not None else 0
                if n > 1:
                    waits = list(si.on_wait)
                    for j, w in enumerate(waits[:-1]):
                        nop = mybir.InstNoOp(
                            name=f"{ins.name}-hw{j}", engine=ins.engine,
                            ins=[], outs=[],
                            sync_info=bass_rust.SyncInfo(on_wait=[w],
                                                         on_update=[]))
                        nc.register_instruction(nop, overwrite=True)
                        newlist.append(nop)
                    si.on_wait = waits[-1:]
                newlist.append(ins)
            blk.instructions = newlist

B, S, D = 2, 2048, 4096
NQ, NKV, DK = 32, 4, 128
NCORES = 8
HPC = NQ // NCORES          # 4 q heads per core
DKC = HPC * DK              # 512 proj cols per core
SCALE = 1.0 / float(np.sqrt(DK))
MV = 30.0                   # masked logit magnitude (post-scale)
EXP_M = float(np.exp(-MV))
QB = 512                    # q block (matmul moving free dim)
KC = 128                    # k chunk (PE contraction/partition dim)
F32 = mybir.dt.float32
BF16 = mybir.dt.bfloat16
EXP = mybir.ActivationFunctionType.Exp


def build_program(s=S):
    """Build the per-core Bass/Tile program. Same program for all 8 cores
    (SPMD); per-core weight slices are supplied via the input maps."""
    nqb = s // QB            # q blocks
    nkc = s // KC            # k chunks
    nd = D // KC             # D contraction chunks (32)
    nnb = D // QB            # 8 column blocks of Wo

    nc = bass.Bass("TRN2", target_bir_lowering=False, debug=False,
                   num_devices=NCORES)
    xbt = nc.dram_tensor("xbt", [B, D, s], BF16, kind="ExternalInput").ap()
    wq = nc.dram_tensor("wq", [D, DKC], BF16, kind="ExternalInput").ap()
    wk = nc.dram_tensor("wk", [D, DK], BF16, kind="ExternalInput").ap()
    wv = nc.dram_tensor("wv", [D, DK], BF16, kind="ExternalInput").ap()
    wo = nc.dram_tensor("wo", [DKC, D], BF16, kind="ExternalInput").ap()
    m01 = nc.dram_tensor("mask01", [4, KC, QB], BF16, kind="ExternalInput").ap()
    rbv = nc.dram_tensor("rbv", [KC, QB], F32, kind="ExternalInput").ap()
    out = nc.dram_tensor("out", [B, s, D], F32, kind="ExternalOutput").ap()

    of = out.rearrange("b s d -> (b s) d")

    with tile.TileContext(nc) as tc, ExitStack() as ctx:
        consts = ctx.enter_context(tc.tile_pool(name="consts", bufs=1))
        ident = consts.tile([128, 128], BF16, name="ident", tag="ident")
        make_identity(nc, ident)
        ones = consts.tile([128, 128], BF16, name="ones", tag="ones")
        nc.vector.memset(ones, 1.0)

        # masks (bf16, applied post-exp)
        m01_t = consts.tile([128, 4, QB], BF16, name="m01_t", tag="m01_t")
        nc.sync.dma_start(out=m01_t, in_=m01.rearrange("d p n -> p d n"))
        # denominator ramp for the last q block: (nskip*128 + c + 1)*exp(-30)
        rbv_t = consts.tile([128, QB], F32, name="rbv_t", tag="rbv_t")
        nc.sync.dma_start(out=rbv_t, in_=rbv)
        zt = consts.tile([128, QB], F32, name="zt", tag="zt")
        nc.vector.memset(zt, 0.0)

        # weights: loaded once, reused for both batches
        wpool = ctx.enter_context(tc.tile_pool(name="wqkv", bufs=1))
        wq_t = wpool.tile([128, nd, DKC], BF16, name="wq_t", tag="wq_t")
        nc.sync.dma_start(out=wq_t, in_=wq.rearrange("(c p) n -> p c n", p=128))
        wk_t = wpool.tile([128, nd, DK], BF16, name="wk_t", tag="wk_t")
        nc.sync.dma_start(out=wk_t, in_=wk.rearrange("(c p) n -> p c n", p=128))
        wv_t = wpool.tile([128, nd, DK], BF16, name="wv_t", tag="wv_t")
        nc.sync.dma_start(out=wv_t, in_=wv.rearrange("(c p) n -> p c n", p=128))
        wo_t = wpool.tile([128, HPC, nnb, QB], BF16, name="wo_t", tag="wo_t")
        # wo_t's DMA is issued after the first projection block (it is only
        # needed in the Wo phase) to keep startup HBM bandwidth for x loads

        # output staging: hoisted (stable address) so the next batch's xq
        # allocation never aliases it and waits on store DMAs
        stpool = ctx.enter_context(tc.tile_pool(name="ostage", bufs=2))
        # x^T loads: hoisted pool (stable buffers -> cross-batch loads only
        # wait on their own previous consumer, not on batch-tail aliases)
        xqp = ctx.enter_context(tc.tile_pool(name="xqld", bufs=2))

        nskip = 4 * (nqb - 1)   # fully-masked chunks of the last q block

        for b in range(B):
            with ExitStack() as bctx:
                bpool = bctx.enter_context(tc.tile_pool(name=f"bp{b}", bufs=1))
                qt = [bpool.tile([128, s], BF16, name=f"qt{b}_{h}", tag=f"qt{h}")
                      for h in range(HPC)]
                kt = bpool.tile([128, s], BF16, name=f"kt{b}", tag="kt")
                vt = bpool.tile([128, s], BF16, name=f"vt{b}", tag="vt")
                vn = bpool.tile([128, s], BF16, name=f"vn{b}", tag="vn")
                cv = bpool.tile([128, 1], F32, name=f"cv{b}", tag="cv")

                # ---------- projection phase: Q^T, K^T, V^T ----------
                # x^T comes pre-transposed from the host (xbt = [B, D, s]);
                # one bulk DMA per q block feeds 192 dependency-free matmuls.
                xbtr = xbt[b].rearrange("(c p) s -> p c s", p=128)
                ndq4 = nd // 4       # D chunks per quarter-load
                with ExitStack() as pctx:
                    qpool = pctx.enter_context(
                        tc.tile_pool(name="qpsum", bufs=1, space="PSUM"))
                    kvpool = pctx.enter_context(
                        tc.tile_pool(name="kvpsum", bufs=1, space="PSUM"))

                    for qb in range(nqb):
                        sl = slice(qb * QB, (qb + 1) * QB)
                        pq = [qpool.tile([128, QB], F32, name=f"pq{h}", tag=f"pq{h}")
                              for h in range(HPC)]
                        pk = kvpool.tile([128, QB], F32, name="pk", tag="pk")
                        pv = kvpool.tile([128, QB], F32, name="pv", tag="pv")
                        for dqi in range(4):
                            xq = xqp.tile([128, ndq4, QB], BF16, name="xq",
                                          tag="xq")
                            # gpsimd SWDGE queue: its stream is free by
                            # mid-attention, so next-batch loads enqueue
                            # early instead of behind the store flood
                            nc.gpsimd.dma_start(
                                out=xq,
                                in_=xbtr[:, dqi * ndq4:(dqi + 1) * ndq4, sl])
                            for dj in range(ndq4):
                                dc = dqi * ndq4 + dj
                                st = dc == 0
                                sp = dc == nd - 1
                                for h in range(HPC):
                                    nc.tensor.matmul(
                                        pq[h], wq_t[:, dc, h * 128:(h + 1) * 128],
                                        xq[:, dj, :], start=st, stop=sp)
                                nc.tensor.matmul(pk, wk_t[:, dc, :],
                                                 xq[:, dj, :], start=st, stop=sp)
                                nc.tensor.matmul(pv, wv_t[:, dc, :],
                                                 xq[:, dj, :], start=st, stop=sp)
                        for h in range(HPC):
                            if h % 2 == 0:
                                nc.vector.tensor_copy(qt[h][:, sl], pq[h])
                            else:
                                nc.scalar.copy(qt[h][:, sl], pq[h])
                        nc.vector.tensor_copy(kt[:, sl], pk)
                        nc.scalar.copy(vt[:, sl], pv)
                    if b == 0:
                        nc.sync.dma_start(
                            out=wo_t,
                            in_=wo.rearrange("(c p) (nb n) -> p c nb n",
                                             p=128, n=QB))

                # ---------- V^T -> V natural; cv = exp(-30)*cumsum(V) ------
                with ExitStack() as vctx:
                    vpsum = vctx.enter_context(
                        tc.tile_pool(name="vtpsum", bufs=2, space="PSUM"))
                    for kc in range(nkc):
                        pvt = vpsum.tile([128, 128], BF16, name="pvt", tag="pvt")
                        nc.tensor.transpose(
                            pvt, vt[:, kc * 128:(kc + 1) * 128], ident)
                        nc.vector.tensor_copy(vn[:, kc * 128:(kc + 1) * 128], pvt)
                    if nskip > 0:
                        cps = vctx.enter_context(
                            tc.tile_pool(name="cvpsum", bufs=1, space="PSUM"))
                        pc = cps.tile([128, 8], F32, name="pc", tag="pc")
                        for i in range(nskip):
                            nc.tensor.matmul(
                                pc, vn[:, i * 128:(i + 1) * 128], ones[:, 0:8],
                                start=(i == 0), stop=(i == nskip - 1))
                        nc.scalar.mul(cv, pc[:, 0:1], EXP_M)
                # cvv[dk, q] = exp(-30) * cumsum_k(V) + cv: the analytic
                # masked-entry numerator for the last q block (replaces the
                # per-chunk exp(-30) mask-adds on GpSimd)
                cumv = bpool.tile([128, QB], F32, name=f"cumv{b}", tag="cumv")
                lsl = slice((nqb - 1) * QB, nqb * QB)
                nc.vector.tensor_tensor_scan(
                    cumv, vt[:, lsl], zt, 0.0,
                    mybir.AluOpType.add, mybir.AluOpType.add)
                cvv = bpool.tile([128, QB], F32, name=f"cvv{b}", tag="cvv")
                nc.vector.tensor_scalar(
                    cvv, cumv, EXP_M, cv,
                    mybir.AluOpType.mult, mybir.AluOpType.add)

                # ---------- attention + output projection (fused) ----------
                # qb-major: all 4 heads of a q block finish together, so the
                # Wo matmuls for that block can start one group behind the
                # attention stream. PSUM: scores 2 + po/pr 4 + Wo 2 = 8 banks.
                apool = bctx.enter_context(tc.tile_pool(name=f"att{b}", bufs=1))
                att = [apool.tile([128, s], BF16, name=f"att{b}_{h}", tag=f"att{h}")
                       for h in range(HPC)]
                qb_order = [nqb - 1] + list(range(nqb - 1))
                with ExitStack() as actx:
                    aps = actx.enter_context(
                        tc.tile_pool(name="atpsum", bufs=2, space="PSUM"))
                    sps = actx.enter_context(
                        tc.tile_pool(name="scpsum", bufs=2, space="PSUM"))
                    opsum = actx.enter_context(
                        tc.tile_pool(name="opsum", bufs=2, space="PSUM"))
                    spool = actx.enter_context(tc.tile_pool(name="attsb", bufs=2))
                    ptp2 = actx.enter_context(tc.tile_pool(name="ptsb", bufs=6))

                    for qb in qb_order:
                        last = qb == nqb - 1
                        qsl = slice(qb * QB, (qb + 1) * QB)
                        kcs = list(range(4 * qb, nkc))
                        if last:
                            # all chunks diagonal: widest first so PSUM
                            # columns initialize before partial accumulation;
                            # masked entries' exp(-30) terms come analytically
                            # via cvv (numerator) and rbv (denominator)
                            chunks = [(kcs[3 - j], (4 - j) * KC)
                                      for j in range(4)]
                        else:
                            # off-diagonal (full-width) chunks first so the
                            # truncated diagonal chunks accumulate onto
                            # initialized PSUM columns. Diagonal chunk d has
                            # valid entries only in its first (d+1)*KC q
                            # columns; the rest are exact zeros post-mask, so
                            # skipping them is exact.
                            chunks = ([(kc, QB) for kc in kcs[4:]]
                                      + [(kcs[d], (d + 1) * KC)
                                         for d in range(4)])
                        for h in range(HPC):
                            po = aps.tile([128, QB], F32, name="po", tag="po")
                            pr = aps.tile([128, QB], F32, name="pr", tag="pr")
                            for i, (kc, w) in enumerate(chunks):
                                q0 = qb * QB
                                ps = sps.tile([128, QB], F32, name="ps", tag="ps")
                                nc.tensor.matmul(
                                    ps[:, 0:w], kt[:, kc * 128:(kc + 1) * 128],
                                    qt[h][:, q0:q0 + w], start=True, stop=True)
                                pt = ptp2.tile([128, QB], BF16, name="pt",
                                               tag="pt")
                                nc.scalar.activation(pt[:, 0:w], ps[:, 0:w],
                                                     EXP, scale=SCALE)
                                d = kc - 4 * qb
                                if d < 4:
                                    # POOL: keeps DVE free for reciprocal
                                    nc.gpsimd.tensor_mul(pt[:, 0:w], pt[:, 0:w],
                                                         m01_t[:, d, 0:w])
                                nc.tensor.matmul(
                                    po[:, 0:w],
                                    vn[:, kc * 128:(kc + 1) * 128], pt[:, 0:w],
                                    start=(i == 0),
                                    stop=(i == len(chunks) - 1))
                                nc.tensor.matmul(
                                    pr[:, 0:w], ones, pt[:, 0:w],
                                    start=(i == 0),
                                    stop=(i == len(chunks) - 1))
                            # drain po/pr to SBUF fast (frees PSUM banks in
                            # ~1us); reciprocal runs later, off the PE path
                            pos = spool.tile([128, QB], F32, name="pos", tag="pos")
                            prs = spool.tile([128, QB], F32, name="prs", tag="prs")
                            nc.scalar.copy(pos, po)
                            if last and nskip > 0:
                                nc.vector.tensor_add(prs, pr, rbv_t)
                            else:
                                nc.vector.tensor_copy(prs, pr)
                            rr = spool.tile([128, QB], F32, name="rr", tag="rr")
                            nc.vector.reciprocal(rr, prs)
                            if last and nskip > 0:
                                tno = spool.tile([128, QB], F32, name="tno",
                                                 tag="tno")
                                nc.vector.tensor_add(tno, pos, cvv)
                                nc.vector.tensor_mul(att[h][:, qsl], tno, rr)
                            else:
                                nc.vector.tensor_mul(att[h][:, qsl], pos, rr)

                    # Wo, chasing the attention stream one q block behind
                    for qb in qb_order:
                        for qti in range(qb * 4, qb * 4 + 4):
                            row0 = b * s + qti * 128
                            for hf in range(2):
                                stg = stpool.tile([128, D // 2], F32,
                                                  name="stg", tag="stg")
                                for nj in range(nnb // 2):
                                    nb = hf * (nnb // 2) + nj
                                    po2 = opsum.tile([128, QB], F32, name="po2",
                                                     tag="po2")
                                    for c in range(HPC):
                                        nc.tensor.matmul(
                                            po2,
                                            att[c][:, qti * 128:(qti + 1) * 128],
                                            wo_t[:, c, nb, :],
                                            start=(c == 0), stop=(c == HPC - 1))
                                    osl = slice(nj * QB, (nj + 1) * QB)
                                    if nb % 2 == 0:
                                        nc.vector.tensor_copy(stg[:, osl], po2)
                                    else:
                                        nc.scalar.copy(stg[:, osl], po2)
                                # column-split stores spread across DMA queues
                                for oc in range(2):
                                    co = hf * (D // 2) + oc * (D // 4)
                                    nc.sync.dma_start(
                                        out=of[row0:row0 + 128,
                                               co:co + D // 4],
                                        in_=stg[:, oc * (D // 4):
                                                (oc + 1) * (D // 4)])
    _split_multiwaits(nc)
    return nc


def make_masks():
    import ml_dtypes

    bf = ml_dtypes.bfloat16
    r = np.arange(KC)[:, None]
    c = np.arange(QB)[None, :]
    valid = [(r + 128 * d) > c for d in range(4)]   # k > q within block
    m01 = np.stack([v.astype(np.float32) for v in valid]).astype(bf)
    nskip = 4 * (S // QB - 1)
    ramp = (nskip * 128 + np.arange(QB) + 1.0) * EXP_M
    rbv = np.broadcast_to(ramp, (KC, QB)).astype(np.float32)
    return m01, np.ascontiguousarray(rbv)


_PROG = {}


def _get_program(s=S):
    if s not in _PROG:
        _PROG[s] = build_program(s)
    return _PROG[s]


def core_in_map(c, x, Wq, Wk, Wv, Wo, _shared={}):
    import ml_dtypes

    bf = ml_dtypes.bfloat16
    xid = id(x)
    if _shared.get("xid") != xid:
        _shared["xid"] = xid
        _shared["xbt"] = np.ascontiguousarray(
            np.asarray(x, dtype=np.float32).astype(bf).transpose(0, 2, 1))
        _shared["m01"], _shared["rbv"] = make_masks()
    h0 = c * HPC
    kv = (c * HPC) // (NQ // NKV)
    return {
        "xbt": _shared["xbt"],
        "wq": np.ascontiguousarray(
            np.asarray(Wq, np.float32)[:, h0 * DK:(h0 + HPC) * DK].astype(bf)),
        "wk": np.ascontiguousarray(
            np.asarray(Wk, np.float32)[:, kv * DK:(kv + 1) * DK].astype(bf)),
        "wv": np.ascontiguousarray(
            np.asarray(Wv, np.float32)[:, kv * DK:(kv + 1) * DK].astype(bf)),
        "wo": np.ascontiguousarray(
            np.asarray(Wo, np.float32)[h0 * DK:(h0 + HPC) * DK, :].astype(bf)),
        "mask01": _shared["m01"],
        "rbv": _shared["rbv"],
    }


def kernel(x, Wq, Wk, Wv, Wo, **kw):
    from concourse.bass_utils import run_bass_kernel_spmd

    nc = _get_program(np.asarray(x).shape[1])
    in_maps = [core_in_map(c, x, Wq, Wk, Wv, Wo) for c in range(NCORES)]
    res = run_bass_kernel_spmd(nc, in_maps, core_ids=list(range(NCORES)), **kw)
    acc = np.zeros(np.asarray(x).shape, np.float64)
    for r in res.results:
        acc += r["out"]
    return acc.astype(np.float32)
